# revision 1
# baseline (speedup 1.0000x reference)
"""Grouped per-sample MLP (conv1d groups=B) + GroupSwish + softmax, on 8 NeuronCores.

Data-parallel over the group/batch axis B=256: 32 groups per core.
Per group g: h = W1[g] @ x[g] + b1[g]; GroupSwish; o = W2[g] @ h + b2[g];
softmax over the flattened [C*L] logits.

Device strategy per core (per group, fully unrolled):
  - W1 matmul out[32, 512], contraction X=784 split 6x128 + 16, operands fed
    as float32r (TF32-like, 1 PE cycle/row, HW rounds internally) straight
    from DMA. fp32r matmuls must write PSUM at partition base 0.
  - GroupSwish via tanh (the only ACT table with both tanh and exp):
    (h+b1)*sigmoid(sp*(h+b1)) = ((h+b1)*0.5) * (1 + tanh(sp*(h+b1)/2)).
    The 1/1.1 factor is folded into W2 host-side; sp = softplus(beta) is
    computed on device via exp/ln.
  - Softmax without max-subtraction (logits are O(1)): exp with fused
    per-partition accum, cross-partition sum / broadcast via tiny matmuls
    against ones vectors.
"""

import os
import numpy as np
from contextlib import ExitStack

import concourse.mybir as mybir
import concourse.tile as tile
from concourse import bacc
from concourse.bass_utils import run_bass_kernel_spmd

B, X, Z, C, L = 256, 784, 32, 10, 512
NCORE = 8
GPC = B // NCORE  # 32 groups per core
NCH = 7  # K-chunks: 6*128 + 16
KLAST = X - 6 * 128  # 16
P = 128
F32 = mybir.dt.float32
F32R = mybir.dt.float32r

DEFAULT_CFG = dict(
    x_layout="interleave",  # "interleave": chunk c = rows 128c+p, 2KB runs;
    #                         "contig": one run/partition (uneven 7/6 rows)
    x_engines=("sync",),  # trigger engines for x loads, round-robin by group
    w_engine="sync",
    out_engine="gpsimd",
    const_engine="gpsimd",
    x_bufs=6,
    h_bufs=3,
    s_bufs=3,
    x_split=False,  # split each group's x-main DMA across sync+scalar queues
    x_pair=False,  # load two groups' x per DMA (halves trigger count)
    pipeline=False,  # defer W2 by one quad and softmax-normalize per quad,
    #                  two quads behind, to keep the PE stream stall-free
)

_CACHE: dict = {}


def _eng(nc, name):
    return getattr(nc, name)


def _build(cfg=DEFAULT_CFG):
    if cfg.get("pipeline"):
        return _build_pipelined(cfg)
    nc = bacc.Bacc("TRN2", target_bir_lowering=False, debug=False)

    xg = nc.dram_tensor("xg", [GPC, X, L], F32R, kind="ExternalInput").ap()
    # W1T packed per quad of groups; each partition reads one contiguous
    # 4*7*32*4B run. w1m[gq, p, j, c, z] = W1[4gq+j][z, row(p, c)] where
    # row depends on x_layout (see _marshal).
    w1m = nc.dram_tensor(
        "w1m", [GPC // 4, P, 4, NCH, Z], F32R, kind="ExternalInput"
    ).ap()
    w2t = nc.dram_tensor("w2t", [Z, GPC * C], F32R, kind="ExternalInput").ap()
    b1c = nc.dram_tensor("b1c", [Z, GPC], F32, kind="ExternalInput").ap()
    btc = nc.dram_tensor("btc", [Z, GPC], F32, kind="ExternalInput").ap()
    b2c = nc.dram_tensor("b2c", [C, GPC], F32, kind="ExternalInput").ap()
    out = nc.dram_tensor("out", [GPC, C, L], F32, kind="ExternalOutput").ap()

    with tile.TileContext(nc) as tc, ExitStack() as ctx:
        consts = ctx.enter_context(tc.tile_pool(name="consts", bufs=1))
        xpool = ctx.enter_context(tc.tile_pool(name="x", bufs=cfg["x_bufs"]))
        wpool = ctx.enter_context(tc.tile_pool(name="w1", bufs=3))
        spool = ctx.enter_context(tc.tile_pool(name="act", bufs=cfg["s_bufs"]))
        hps = ctx.enter_context(
            tc.tile_pool(name="hps", bufs=cfg["h_bufs"], space="PSUM")
        )
        ops = ctx.enter_context(tc.tile_pool(name="ops", bufs=2, space="PSUM"))
        tps = ctx.enter_context(tc.tile_pool(name="tps", bufs=2, space="PSUM"))

        ce = _eng(nc, cfg["const_engine"])
        we = _eng(nc, cfg["w_engine"])
        oe = _eng(nc, cfg["out_engine"])

        # --- constants / per-group scalars ---
        w2tt = consts.tile([Z, GPC * C], F32R, name="w2tt")
        ce.dma_start(w2tt[:], w2t)
        b1t = consts.tile([Z, GPC], F32, name="b1t")
        ce.dma_start(b1t[:], b1c)
        btt = consts.tile([Z, GPC], F32, name="btt")
        ce.dma_start(btt[:], btc)
        b2t = consts.tile([C, GPC], F32, name="b2t")
        ce.dma_start(b2t[:], b2c)
        ones_k = consts.tile([C, 1], F32, name="ones_k")
        nc.vector.memset(ones_k[:], 1.0)
        ones_m = consts.tile([1, C], F32, name="ones_m")
        nc.vector.memset(ones_m[:], 1.0)

        # sp = softplus(beta) = ln(1 + exp(beta)); halves for tanh-sigmoid
        spe = consts.tile([Z, GPC], F32, name="spe")
        nc.scalar.activation(spe[:], btt[:], mybir.ActivationFunctionType.Exp)
        spe1 = consts.tile([Z, GPC], F32, name="spe1")
        nc.vector.tensor_scalar_add(spe1[:], spe[:], 1.0)
        spt = consts.tile([Z, GPC], F32, name="spt")
        nc.scalar.activation(spt[:], spe1[:], mybir.ActivationFunctionType.Ln)
        sph = consts.tile([Z, GPC], F32, name="sph")
        nc.vector.tensor_scalar_mul(sph[:], spt[:], 0.5)
        spb1h = consts.tile([Z, GPC], F32, name="spb1h")
        nc.vector.tensor_mul(spb1h[:], sph[:], b1t[:])

        xt2 = None
        for g in range(GPC):
            gq, jq = divmod(g, 4)
            xe = _eng(nc, cfg["x_engines"][g % len(cfg["x_engines"])])
            if cfg["x_pair"]:
                # one [P, 2*7*L] tile per pair of groups; group g%2==i owns
                # free columns [i*NCH*L, (i+1)*NCH*L) logically remapped below
                if g % 2 == 0:
                    xt2 = xpool.tile([P, 2 * NCH * L], F32R, tag="xt", name=f"xt{g}")
                    xe.dma_start(
                        xt2[:, : 12 * L].rearrange("p (i c l) -> p i c l", i=2, c=6),
                        xg[g : g + 2, : 6 * P].rearrange("i (c p) l -> p i c l", p=P),
                    )
                    xe.dma_start(
                        xt2[:KLAST, 12 * L :].rearrange("p (i l) -> p i l", i=2),
                        xg[g : g + 2, 6 * P :].rearrange("i r l -> r i l"),
                    )
                i = g % 2
                xt = xt2[:, i * 6 * L : (i + 1) * 6 * L]
                xlast = xt2[:, (12 + i) * L : (13 + i) * L]
            elif cfg["x_layout"] == "interleave":
                # chunk c = rows 128c..128c+128; 2KB runs across partitions
                xt = xpool.tile([P, NCH * L], F32R, tag="xt", name=f"xt{g}")
                xlast = xt[:, 6 * L :]
                if cfg["x_split"]:
                    nc.sync.dma_start(
                        xt[:, : 3 * L].rearrange("p (c l) -> p c l", c=3),
                        xg[g, : 3 * P].rearrange("(c p) l -> p c l", p=P),
                    )
                    nc.scalar.dma_start(
                        xt[:, 3 * L : 6 * L].rearrange("p (c l) -> p c l", c=3),
                        xg[g, 3 * P : 6 * P].rearrange("(c p) l -> p c l", p=P),
                    )
                else:
                    xe.dma_start(
                        xt[:, : 6 * L].rearrange("p (c l) -> p c l", c=6),
                        xg[g, : 6 * P].rearrange("(c p) l -> p c l", p=P),
                    )
                xe.dma_start(xt[:KLAST, 6 * L :], xg[g, 6 * P :])
            else:
                # one contiguous run per partition: p<16 -> rows 7p..7p+7,
                # p>=16 -> rows 112+6(p-16)..+6
                xt = xpool.tile([P, NCH * L], F32R, tag="xt", name=f"xt{g}")
                xlast = xt[:, 6 * L :]
                xe.dma_start(
                    xt[:16, :].rearrange("p (c l) -> p c l", c=NCH),
                    xg[g, : 7 * 16].rearrange("(p c) l -> p c l", p=16),
                )
                xe.dma_start(
                    xt[16:, : 6 * L].rearrange("p (c l) -> p c l", c=6),
                    xg[g, 7 * 16 : X].rearrange("(p c) l -> p c l", p=112),
                )
            # --- W1T for a quad of 4 groups, one DMA every 4th group ---
            if jq == 0:
                wt = wpool.tile([P, 4 * NCH * Z], F32R, tag="wt", name=f"wt{g}")
                we.dma_start(
                    wt[:].rearrange("p (j c z) -> p j c z", j=4, c=NCH),
                    w1m[gq],
                )

            # --- h = W1 @ x ---
            h = hps.tile([Z, L], F32, tag="h", name=f"h{g}")
            for c in range(NCH):
                kk = P if c < 6 else KLAST
                rhs = (
                    xt[:, c * L : (c + 1) * L] if c < 6 else xlast[:KLAST, :]
                )
                nc.tensor.matmul(
                    h[:],
                    wt[:kk, (jq * NCH + c) * Z : (jq * NCH + c + 1) * Z],
                    rhs,
                    start=(c == 0),
                    stop=(c == NCH - 1),
                )

            # --- GroupSwish: ((h+b1)*0.5) * (1 + tanh(sp*(h+b1)/2)) ---
            t = spool.tile([Z, L], F32, tag="t", name=f"t{g}")
            nc.scalar.activation(
                t[:],
                h[:],
                mybir.ActivationFunctionType.Tanh,
                bias=spb1h[:, g : g + 1],
                scale=sph[:, g : g + 1],
            )
            u = spool.tile([Z, L], F32, tag="u", name=f"u{g}")
            nc.vector.tensor_scalar(
                u[:],
                h[:],
                b1t[:, g : g + 1],
                0.5,
                op0=mybir.AluOpType.add,
                op1=mybir.AluOpType.mult,
            )
            swish = spool.tile([Z, L], F32R, tag="swish", name=f"sw{g}")
            nc.vector.scalar_tensor_tensor(
                swish[:],
                t[:],
                1.0,
                u[:],
                op0=mybir.AluOpType.add,
                op1=mybir.AluOpType.mult,
            )

            # --- o = (W2/1.1) @ swish ---
            o = ops.tile([C, L], F32, tag="o", name=f"o{g}")
            nc.tensor.matmul(
                o[:], w2tt[:, g * C : (g + 1) * C], swish[:], start=True, stop=True
            )

            # --- softmax over [C, L] (no max subtraction) ---
            expo = spool.tile([C, L], F32, tag="expo", name=f"e{g}")
            esum = spool.tile([C, 1], F32, tag="esum", name=f"es{g}")
            nc.scalar.activation(
                expo[:],
                o[:],
                mybir.ActivationFunctionType.Exp,
                bias=b2t[:, g : g + 1],
                scale=1.0,
                accum_out=esum[:],
            )
            tot = tps.tile([1, 1], F32, tag="tb", name=f"tot{g}")
            nc.tensor.matmul(tot[:], ones_k[:], esum[:], start=True, stop=True)
            inv = spool.tile([1, 1], F32, tag="inv", name=f"inv{g}")
            nc.vector.reciprocal(inv[:], tot[:])
            bc = tps.tile([C, 1], F32, tag="tb", name=f"bc{g}")
            nc.tensor.matmul(bc[:], ones_m[:], inv[:], start=True, stop=True)
            invc = spool.tile([C, 1], F32, tag="invc", name=f"ic{g}")
            nc.vector.tensor_copy(invc[:], bc[:])
            res = spool.tile([C, L], F32, tag="res", name=f"r{g}")
            nc.vector.tensor_scalar_mul(res[:], expo[:], invc[:])

            oe.dma_start(out[g], res[:])

    nc.compile()
    return nc


def _build_pipelined(cfg):
    """Software-pipelined emission: the PE stream per quad q is
    [28x W1(q)] [4x W2(q-1)] [tot4(q-2), bc4(q-2)] so every cross-engine
    dependency (swish from DVE, exp sums from ACT, reciprocal from DVE) has
    a full quad of slack before the PE needs it."""
    nc = bacc.Bacc("TRN2", target_bir_lowering=False, debug=False)
    NQ = GPC // 4

    xg = nc.dram_tensor("xg", [GPC, X, L], F32R, kind="ExternalInput").ap()
    w1m = nc.dram_tensor(
        "w1m", [NQ, P, 4, NCH, Z], F32R, kind="ExternalInput"
    ).ap()
    w2t = nc.dram_tensor("w2t", [Z, GPC * C], F32R, kind="ExternalInput").ap()
    b1c = nc.dram_tensor("b1c", [Z, GPC], F32, kind="ExternalInput").ap()
    btc = nc.dram_tensor("btc", [Z, GPC], F32, kind="ExternalInput").ap()
    b2c = nc.dram_tensor("b2c", [C, GPC], F32, kind="ExternalInput").ap()
    out = nc.dram_tensor("out", [GPC, C, L], F32, kind="ExternalOutput").ap()

    with tile.TileContext(nc) as tc, ExitStack() as ctx:
        consts = ctx.enter_context(tc.tile_pool(name="consts", bufs=1))
        xpool = ctx.enter_context(tc.tile_pool(name="x", bufs=cfg["x_bufs"]))
        wpool = ctx.enter_context(tc.tile_pool(name="w1", bufs=3))
        spool = ctx.enter_context(tc.tile_pool(name="act", bufs=cfg["s_bufs"]))
        dpool = ctx.enter_context(tc.tile_pool(name="deep", bufs=10))
        e4pool = ctx.enter_context(tc.tile_pool(name="e4", bufs=3))
        hps = ctx.enter_context(
            tc.tile_pool(name="hps", bufs=cfg["h_bufs"], space="PSUM")
        )
        ops = ctx.enter_context(tc.tile_pool(name="ops", bufs=2, space="PSUM"))
        tps = ctx.enter_context(tc.tile_pool(name="tps", bufs=2, space="PSUM"))

        oe = _eng(nc, cfg["out_engine"])
        ce = _eng(nc, cfg["const_engine"])
        we = _eng(nc, cfg["w_engine"])

        w2tt = consts.tile([Z, GPC * C], F32R, name="w2tt")
        ce.dma_start(w2tt[:], w2t)
        b1t = consts.tile([Z, GPC], F32, name="b1t")
        ce.dma_start(b1t[:], b1c)
        btt = consts.tile([Z, GPC], F32, name="btt")
        ce.dma_start(btt[:], btc)
        b2t = consts.tile([C, GPC], F32, name="b2t")
        ce.dma_start(b2t[:], b2c)
        ones_k = consts.tile([C, 1], F32, name="ones_k")
        nc.vector.memset(ones_k[:], 1.0)
        ones_m = consts.tile([1, C], F32, name="ones_m")
        nc.vector.memset(ones_m[:], 1.0)

        spe = consts.tile([Z, GPC], F32, name="spe")
        nc.scalar.activation(spe[:], btt[:], mybir.ActivationFunctionType.Exp)
        spe1 = consts.tile([Z, GPC], F32, name="spe1")
        nc.vector.tensor_scalar_add(spe1[:], spe[:], 1.0)
        spt = consts.tile([Z, GPC], F32, name="spt")
        nc.scalar.activation(spt[:], spe1[:], mybir.ActivationFunctionType.Ln)
        sph = consts.tile([Z, GPC], F32, name="sph")
        nc.vector.tensor_scalar_mul(sph[:], spt[:], 0.5)
        spb1h = consts.tile([Z, GPC], F32, name="spb1h")
        nc.vector.tensor_mul(spb1h[:], sph[:], b1t[:])

        swishes = {}  # g -> tile
        expos = {}  # g -> tile
        esums = {}  # q -> [C, 4] tile
        n_x = len(cfg["x_engines"])

        def stage1(q):
            """x/w loads, W1 matmuls, swish for quad q."""
            wt = wpool.tile([P, 4 * NCH * Z], F32R, tag="wt", name=f"wt{q}")
            we.dma_start(
                wt[:].rearrange("p (j c z) -> p j c z", j=4, c=NCH), w1m[q]
            )
            for j in range(4):
                g = 4 * q + j
                xe = _eng(nc, cfg["x_engines"][g % n_x])
                xt = xpool.tile([P, NCH * L], F32R, tag="xt", name=f"xt{g}")
                if cfg["x_split"]:
                    nc.sync.dma_start(
                        xt[:, : 3 * L].rearrange("p (c l) -> p c l", c=3),
                        xg[g, : 3 * P].rearrange("(c p) l -> p c l", p=P),
                    )
                    nc.scalar.dma_start(
                        xt[:, 3 * L : 6 * L].rearrange("p (c l) -> p c l", c=3),
                        xg[g, 3 * P : 6 * P].rearrange("(c p) l -> p c l", p=P),
                    )
                else:
                    xe.dma_start(
                        xt[:, : 6 * L].rearrange("p (c l) -> p c l", c=6),
                        xg[g, : 6 * P].rearrange("(c p) l -> p c l", p=P),
                    )
                xe.dma_start(xt[:KLAST, 6 * L :], xg[g, 6 * P :])

                h = hps.tile([Z, L], F32, tag="h", name=f"h{g}")
                for c in range(NCH):
                    kk = P if c < 6 else KLAST
                    nc.tensor.matmul(
                        h[:],
                        wt[:kk, (j * NCH + c) * Z : (j * NCH + c + 1) * Z],
                        xt[:kk, c * L : (c + 1) * L],
                        start=(c == 0),
                        stop=(c == NCH - 1),
                    )
                t = spool.tile([Z, L], F32, tag="t", name=f"t{g}")
                nc.scalar.activation(
                    t[:],
                    h[:],
                    mybir.ActivationFunctionType.Tanh,
                    bias=spb1h[:, g : g + 1],
                    scale=sph[:, g : g + 1],
                )
                u = spool.tile([Z, L], F32, tag="u", name=f"u{g}")
                nc.vector.tensor_scalar(
                    u[:],
                    h[:],
                    b1t[:, g : g + 1],
                    0.5,
                    op0=mybir.AluOpType.add,
                    op1=mybir.AluOpType.mult,
                )
                sw = dpool.tile([Z, L], F32R, tag="swish", name=f"sw{g}")
                nc.vector.scalar_tensor_tensor(
                    sw[:],
                    t[:],
                    1.0,
                    u[:],
                    op0=mybir.AluOpType.add,
                    op1=mybir.AluOpType.mult,
                )
                swishes[g] = sw

        def stage2(q):
            """W2 matmuls + exp for quad q (emitted one quad later)."""
            esum4 = e4pool.tile([C, 4], F32, tag="esum4", name=f"es4_{q}")
            esums[q] = esum4
            for j in range(4):
                g = 4 * q + j
                o = ops.tile([C, L], F32, tag="o", name=f"o{g}")
                nc.tensor.matmul(
                    o[:],
                    w2tt[:, g * C : (g + 1) * C],
                    swishes.pop(g)[:],
                    start=True,
                    stop=True,
                )
                expo = dpool.tile([C, L], F32, tag="expo", name=f"e{g}")
                nc.scalar.activation(
                    expo[:],
                    o[:],
                    mybir.ActivationFunctionType.Exp,
                    bias=b2t[:, g : g + 1],
                    scale=1.0,
                    accum_out=esum4[:, j : j + 1],
                )
                expos[g] = expo

        def stage3(q):
            """Normalization + store for quad q (emitted two quads later)."""
            esum4 = esums.pop(q)
            tot4 = tps.tile([1, 4], F32, tag="tb", name=f"tot{q}")
            nc.tensor.matmul(tot4[:], ones_k[:], esum4[:], start=True, stop=True)
            inv4 = spool.tile([1, 4], F32, tag="inv", name=f"inv{q}")
            nc.vector.reciprocal(inv4[:], tot4[:])
            bc4 = tps.tile([C, 4], F32, tag="tb", name=f"bc{q}")
            nc.tensor.matmul(bc4[:], ones_m[:], inv4[:], start=True, stop=True)
            invc4 = spool.tile([C, 4], F32, tag="invc", name=f"ic{q}")
            nc.vector.tensor_copy(invc4[:], bc4[:])
            for j in range(4):
                g = 4 * q + j
                res = spool.tile([C, L], F32, tag="res", name=f"r{g}")
                nc.vector.tensor_scalar_mul(
                    res[:], expos.pop(g)[:], invc4[:, j : j + 1]
                )
                oe.dma_start(out[g], res[:])

        for q in range(NQ):
            stage1(q)
            if q >= 1:
                stage2(q - 1)
            if q >= 2:
                stage3(q - 2)
        stage2(NQ - 1)
        stage3(NQ - 2)
        stage3(NQ - 1)

    nc.compile()
    return nc


def _marshal(x, W1, b1, beta, W2, b2, cfg=DEFAULT_CFG):
    """Full inputs -> list of per-core input dicts."""
    xg = np.ascontiguousarray(x, dtype=np.float32).reshape(B, X, L)
    w1T = W1.astype(np.float32, copy=False).transpose(0, 2, 1)  # [B, X, Z]
    w1m = np.zeros((B // 4, P, 4, NCH, Z), np.float32)
    if cfg["x_layout"] == "interleave":
        # w1m[gq, p, j, c, z] = W1T[4gq+j, 128c+p, z]
        main = w1T[:, : 6 * P].reshape(B // 4, 4, 6, P, Z)
        w1m[:, :, :, :6] = main.transpose(0, 3, 1, 2, 4)
        left = w1T[:, 6 * P :].reshape(B // 4, 4, KLAST, Z)
        w1m[:, :KLAST, :, 6] = left.transpose(0, 2, 1, 3)
    else:
        # row(p, c) = 7p+c for p<16, 112+6(p-16)+c for p>=16
        lo = w1T[:, : 7 * 16].reshape(B // 4, 4, 16, NCH, Z)
        hi = w1T[:, 7 * 16 :].reshape(B // 4, 4, 112, 6, Z)
        w1m[:, :16] = lo.transpose(0, 2, 1, 3, 4)
        w1m[:, 16:, :, :6] = hi.transpose(0, 2, 1, 3, 4)
    w2s = (W2.astype(np.float32, copy=False) * np.float32(1.0 / 1.1)).transpose(
        0, 2, 1
    )  # [B, Z, C]

    in_maps = []
    for core in range(NCORE):
        s = slice(core * GPC, (core + 1) * GPC)
        sq = slice(core * GPC // 4, (core + 1) * GPC // 4)
        in_maps.append(
            {
                "xg": xg[s],
                "w1m": w1m[sq],
                # [Z, GPC*C]: w2t[z, g*C+c] = W2[g0+g, c, z] / 1.1
                "w2t": np.ascontiguousarray(
                    w2s[s].transpose(1, 0, 2).reshape(Z, GPC * C)
                ),
                "b1c": np.ascontiguousarray(b1[s].astype(np.float32).T),
                "btc": np.ascontiguousarray(
                    np.broadcast_to(beta[s].astype(np.float32), (Z, GPC))
                ),
                "b2c": np.ascontiguousarray(b2[s].astype(np.float32).T),
            }
        )
    return in_maps


def _run(in_maps, cfg=DEFAULT_CFG, trace=False, tmpdir=None):
    key = str(sorted(cfg.items()))
    if key not in _CACHE:
        _CACHE[key] = _build(cfg)
    return run_bass_kernel_spmd(
        _CACHE[key],
        in_maps,
        core_ids=list(range(NCORE)),
        trace=trace,
        tmpdir=tmpdir,
    )


_LAST = {}


def kernel(x, W1, b1, beta, W2, b2):
    in_maps = _marshal(x, W1, b1, beta, W2, b2)
    trace = bool(os.environ.get("KERNEL_TRACE"))
    r = _run(in_maps, trace=trace, tmpdir=os.environ.get("KERNEL_TRACE_DIR"))
    _LAST["results"] = r
    outs = [r.results[c]["out"].reshape(GPC, C * L) for c in range(NCORE)]
    return np.concatenate(outs, axis=0)



# revision 2
# speedup vs baseline: 1.0068x; 1.0068x over previous
"""Grouped per-sample MLP (conv1d groups=B) + GroupSwish + softmax, 8 NeuronCores.

Data-parallel over B=256: 32 groups/core, processed in quads of 4 groups
stacked on the partition axis so every post-GEMM op is one instruction per
quad:

  h4[32j:32j+32] = W1[g] @ x[g] + b1[g]      (7 fp16 matmuls per group, b1 via
                                              a ones-row in the K=17 tail chunk)
  t4  = tanh(sp/2 * h4 + sp/2*b1)            (one ACT op per quad, per-partition
                                              scale/bias encode the group)
  sw4 = (1 + t4) * h4                        (one DVE op; equals
                                              2*(h+b1)*sigmoid(sp*(h+b1)))
  o4  = W2q[q] @ sw4                         (block-diagonal [128,40] fp16 lhsT,
                                              scaled by 0.5/1.1 host-side)
  expo4, esum4 = exp(o4 + b2)                (one ACT op + free-dim accum)
  softmax: tot4/bc4 via tiny block-ones matmuls, reciprocal, one multiply.

x and W1 are cast to fp16 on host (output rel err ~1e-3, budget 2e-2), which
halves the dominant HBM traffic. x main chunks load as one contiguous
[128 part x 6KB] DMA per group (zero-copy host reshape); the 16-row tail
loads quad-batched. sp = softplus(beta) is computed host-side.
"""

import os
import numpy as np
from contextlib import ExitStack

import concourse.mybir as mybir
import concourse.tile as tile
from concourse import bacc
from concourse.bass_utils import run_bass_kernel_spmd

B, X, Z, C, L = 256, 784, 32, 10, 512
NCORE = 8
GPC = B // NCORE  # 32 groups per core
NQ = GPC // 4  # 8 quads per core
P = 128
XMAIN = 6 * P  # 768 rows in the main chunks
XT = X - XMAIN  # 16 tail rows
F16 = mybir.dt.float16
F32 = mybir.dt.float32
F8 = mybir.dt.float8e3
SX, SW = 2.0, 8.0  # fp8 range scaling: x*SX, W1*SW; folded out downstream

DEFAULT_CFG = dict(
    x_bufs=18,
    xt_bufs=4,
    h_bufs=3,
    s_bufs=4,
    lag=1,
    x_engine="sync",
    w_engine="scalar",
    out_engine="scalar",
)

_CACHE: dict = {}


def _build(cfg=DEFAULT_CFG):
    nc = bacc.Bacc("TRN2", target_bir_lowering=False, debug=False)

    xm = nc.dram_tensor("xm", [GPC, P, 6 * L], F8, kind="ExternalInput").ap()
    xt6 = nc.dram_tensor("xt6", [NQ, 4 * (XT + 1), L], F8, kind="ExternalInput").ap()
    w1tail = nc.dram_tensor("w1tail", [4 * (XT + 1), NQ * P], F8, kind="ExternalInput").ap()
    w1m = nc.dram_tensor("w1m", [P, NQ * 4 * 7 * Z], F8, kind="ExternalInput").ap()
    w2q = nc.dram_tensor("w2q", [P, NQ * 40], F16, kind="ExternalInput").ap()
    sphalf = nc.dram_tensor("sphalf", [P, NQ], F32, kind="ExternalInput").ap()
    spb1h = nc.dram_tensor("spb1h", [P, NQ], F32, kind="ExternalInput").ap()
    b2q = nc.dram_tensor("b2q", [40, NQ], F32, kind="ExternalInput").ap()
    bo1 = nc.dram_tensor("bo1", [40, 4], F32, kind="ExternalInput").ap()
    bo2 = nc.dram_tensor("bo2", [4, 40], F32, kind="ExternalInput").ap()
    out = nc.dram_tensor("out", [NQ, 40, L], F32, kind="ExternalOutput").ap()

    with tile.TileContext(nc) as tc, ExitStack() as ctx:
        consts = ctx.enter_context(tc.tile_pool(name="consts", bufs=1))
        xpool = ctx.enter_context(tc.tile_pool(name="x", bufs=cfg["x_bufs"]))
        xtpool = ctx.enter_context(tc.tile_pool(name="xt", bufs=cfg["xt_bufs"]))
        spool = ctx.enter_context(tc.tile_pool(name="act", bufs=cfg["s_bufs"]))
        epool = ctx.enter_context(tc.tile_pool(name="expo", bufs=NQ))
        hps = ctx.enter_context(
            tc.tile_pool(name="hps", bufs=cfg["h_bufs"], space="PSUM")
        )
        ops = ctx.enter_context(tc.tile_pool(name="ops", bufs=2, space="PSUM"))
        t1ps = ctx.enter_context(tc.tile_pool(name="t1ps", bufs=1, space="PSUM"))
        t2ps = ctx.enter_context(tc.tile_pool(name="t2ps", bufs=1, space="PSUM"))

        xe = getattr(nc, cfg["x_engine"])
        we = getattr(nc, cfg["w_engine"])
        oe = getattr(nc, cfg["out_engine"])

        # --- constants (on the w engine so the x engine starts immediately) ---
        # w1 loads in per-quad column chunks so the first W1 matmul only
        # waits for its own quad's weights
        w1t = consts.tile([P, NQ * 4 * 7 * Z], F8, name="w1t")
        WQC = 4 * 7 * Z
        for q in range(NQ):
            we.dma_start(
                w1t[:, q * WQC : (q + 1) * WQC], w1m[:, q * WQC : (q + 1) * WQC]
            )
        w1tt = consts.tile([4 * (XT + 1), NQ * P], F8, name="w1tt")
        we.dma_start(w1tt[:], w1tail)
        w2t = consts.tile([P, NQ * 40], F16, name="w2t")
        we.dma_start(w2t[:], w2q)
        spht = consts.tile([P, NQ], F32, name="spht")
        we.dma_start(spht[:], sphalf)
        spbt = consts.tile([P, NQ], F32, name="spbt")
        we.dma_start(spbt[:], spb1h)
        b2t = consts.tile([40, NQ], F32, name="b2t")
        we.dma_start(b2t[:], b2q)
        bo1t = consts.tile([40, 4], F32, name="bo1t")
        we.dma_start(bo1t[:], bo1)
        bo2t = consts.tile([4, 40], F32, name="bo2t")
        we.dma_start(bo2t[:], bo2)

        # dummy ACT op so the (single) exp_and_others table load runs during
        # the first x DMA instead of stalling the first real tanh
        warm = consts.tile([1, 16], F32, name="warm")
        nc.vector.memset(warm[:], 0.0)
        warm2 = consts.tile([1, 16], F32, name="warm2")
        nc.scalar.activation(warm2[:], warm[:], mybir.ActivationFunctionType.Tanh)

        # per-quad esums accumulate here
        esumA = consts.tile([40, NQ], F32, name="esumA")
        expos = [None] * NQ
        h4s = [None] * NQ
        sw4s = [None] * NQ
        invs = [None] * NQ

        def stage1a(q):
            """x loads + W1 matmuls for quad q."""
            # tail rows for the 4 groups stacked [68, L]; ones rows for the
            # b1 trick are baked in host-side
            x4 = xtpool.tile([4 * (XT + 1), L], F8, tag="x4", name=f"x4_{q}")
            xe.dma_start(x4[:], xt6[q])
            xts = []
            for j in range(4):
                g = 4 * q + j
                xt = xpool.tile([P, 6 * L], F8, tag="xt", name=f"xt{g}")
                if q == 0:
                    # split halves so the first W1 chunks start ~1us earlier
                    xe.dma_start(xt[:, : 3 * L], xm[g, :, : 3 * L])
                    xe.dma_start(xt[:, 3 * L :], xm[g, :, 3 * L :])
                else:
                    xe.dma_start(xt[:], xm[g])
                xts.append(xt)

            h4 = hps.tile([P, L], F32, tag="h4", name=f"h4_{q}")
            h4s[q] = h4
            # one K=68 block-diagonal matmul handles all 4 groups' tail rows
            # (+ the b1 ones-rows); start=True resets the whole h4 region
            nc.tensor.matmul(
                h4[:],
                w1tt[:, q * P : (q + 1) * P],
                x4[:],
                start=True,
                stop=False,
            )
            for j in range(4):
                base = ((q * 4 + j) * 7) * Z
                for c in range(6):
                    nc.tensor.matmul(
                        h4[32 * j : 32 * (j + 1), :],
                        w1t[:, base + c * Z : base + (c + 1) * Z],
                        xts[j][:, c * L : (c + 1) * L],
                        start=False,
                        stop=(c == 5),
                        tile_position=(0, 32 * j),
                    )

        def stage1b(q):
            """tanh + swish for quad q (emitted after stage2(q-1) so the
            w2g gate op outranks sw4 in DVE priority)."""
            h4 = h4s[q]
            # GroupSwish: sw4 = (1 + tanh(sp/2*h')) * h'
            t4 = spool.tile([P, L], F16, tag="t4", name=f"t4_{q}")
            # h4 already includes b1 (ones-row chunk), so no bias here
            nc.scalar.activation(
                t4[:],
                h4[:],
                mybir.ActivationFunctionType.Tanh,
                bias=0.0,
                scale=spht[:, q : q + 1],
            )
            sw4 = spool.tile([P, L], F16, tag="sw4", name=f"sw4_{q}")
            sw4s[q] = sw4
            nc.vector.scalar_tensor_tensor(
                sw4[:],
                t4[:],
                1.0,
                h4[:],
                op0=mybir.AluOpType.add,
                op1=mybir.AluOpType.mult,
            )

        def stage2(q, tok):
            """W2 + exp for quad q; tok (h4 of a later quad) gates the W2
            weight slice so the scheduler cannot glue W2(q) into the
            W1(q)->tanh->sw latency shadow."""
            if tok is not None:
                w2g = spool.tile([P, 40], F16, tag="w2g", name=f"w2g_{q}")
                nc.vector.scalar_tensor_tensor(
                    w2g[:],
                    tok[:, :40],
                    0.0,
                    w2t[:, q * 40 : (q + 1) * 40],
                    op0=mybir.AluOpType.mult,
                    op1=mybir.AluOpType.add,
                )
                lhs = w2g[:]
            else:
                lhs = w2t[:, q * 40 : (q + 1) * 40]
            o4 = ops.tile([40, L], F32, tag="o4", name=f"o4_{q}")
            nc.tensor.matmul(o4[:], lhs, sw4s[q][:], start=True, stop=True)
            expo4 = epool.tile([40, L], F32, tag="expo4", name=f"e4_{q}")
            expos[q] = expo4
            nc.scalar.activation(
                expo4[:],
                o4[:],
                mybir.ActivationFunctionType.Exp,
                bias=b2t[:, q : q + 1],
                scale=1.0,
                accum_out=esumA[:, q : q + 1],
            )

        def stage3a(q, tok):
            """Cross-partition esum: tot4 = blockones^T @ esum; reciprocal.
            tok (h4 of quad q+2) gates the tiny matmul off the W1 stream."""
            if tok is not None:
                bo1g = spool.tile([40, 4], F32, tag="bo1g", name=f"bo1g_{q}")
                nc.vector.scalar_tensor_tensor(
                    bo1g[:],
                    tok[:40, :4],
                    0.0,
                    bo1t[:],
                    op0=mybir.AluOpType.mult,
                    op1=mybir.AluOpType.add,
                )
                lhs = bo1g[:]
            else:
                lhs = bo1t[:]
            tot4 = t1ps.tile([4, 1], F32, tag="tot4", name=f"tot4_{q}")
            nc.tensor.matmul(
                tot4[:], lhs, esumA[:, q : q + 1], start=True, stop=True
            )
            inv4 = spool.tile([4, 1], F32, tag="inv4", name=f"inv4_{q}")
            invs[q] = inv4
            nc.vector.reciprocal(inv4[:], tot4[:])

        def stage3b(q, tok):
            """Broadcast 1/tot to the 40 partitions, normalize, store."""
            if tok is not None:
                bo2g = spool.tile([4, 40], F32, tag="bo2g", name=f"bo2g_{q}")
                nc.vector.scalar_tensor_tensor(
                    bo2g[:],
                    tok[:4, :40],
                    0.0,
                    bo2t[:],
                    op0=mybir.AluOpType.mult,
                    op1=mybir.AluOpType.add,
                )
                lhs = bo2g[:]
            else:
                lhs = bo2t[:]
            bc4 = t2ps.tile([40, 1], F32, tag="bc4", name=f"bc4_{q}")
            nc.tensor.matmul(bc4[:], lhs, invs[q][:], start=True, stop=True)
            invc4 = spool.tile([40, 1], F32, tag="invc4", name=f"ic4_{q}")
            nc.vector.tensor_copy(invc4[:], bc4[:])
            res4 = spool.tile([40, L], F32, tag="res4", name=f"r4_{q}")
            nc.vector.tensor_scalar_mul(res4[:], expos[q][:], invc4[:])
            oe.dma_start(out[q], res4[:])

        lag = cfg["lag"]
        for q in range(NQ):
            stage1a(q)
            if q >= lag:
                stage2(q - lag, h4s[q])
            if q >= lag + 1:
                stage3a(q - lag - 1, h4s[q])
            if q >= lag + 2:
                stage3b(q - lag - 2, h4s[q])
            stage1b(q)
        for k in range(NQ - lag, NQ + 2):
            if k < NQ:
                stage2(k, None)
            if NQ - lag - 1 <= k - 1 < NQ:
                stage3a(k - 1, None)
            if NQ - lag - 2 <= k - 2 < NQ:
                stage3b(k - 2, None)

    nc.compile()
    return nc


def _marshal(x, W1, b1, beta, W2, b2, cfg=DEFAULT_CFG):
    """Full inputs -> list of per-core input dicts."""
    import ml_dtypes
    F8NP = ml_dtypes.float8_e3m4
    x16 = (np.ascontiguousarray(x, dtype=np.float32).reshape(B, X, L) * np.float32(SX)).astype(F8NP)
    w1T16 = (W1.astype(np.float32) * np.float32(SW)).astype(F8NP).transpose(0, 2, 1)  # [B, X, Z]
    b116 = (b1.astype(np.float32) * np.float32(SX * SW)).astype(F8NP)
    sp = np.log1p(np.exp(beta.astype(np.float64))).astype(np.float32)  # softplus
    w2s = (W2.astype(np.float32) * np.float32(0.5 / 1.1 / (SX * SW))).astype(np.float16)

    bo1 = np.zeros((40, 4), np.float32)
    bo2 = np.zeros((4, 40), np.float32)
    for j in range(4):
        bo1[10 * j : 10 * (j + 1), j] = 1.0
        bo2[j, 10 * j : 10 * (j + 1)] = 1.0

    in_maps = []
    for core in range(NCORE):
        s = slice(core * GPC, (core + 1) * GPC)
        xc = x16[s]  # [GPC, X, L]
        xm = xc[:, :XMAIN].reshape(GPC, P, 6 * L)
        xt6 = np.ones((NQ, 4, XT + 1, L), xc.dtype)
        xt6[:, :, :XT] = xc[:, XMAIN:].reshape(NQ, 4, XT, L)
        xt6 = xt6.reshape(NQ, 4 * (XT + 1), L)
        # w1m[p, ((q*4+j)*7+c)*Z + z]:
        #   c<6: W1T[g, 6p+c, z]; c=6: p<16 -> W1T[g, 768+p, z], p=16 -> b1[g,z]
        w1c = w1T16[s]  # [GPC, X, Z]
        w1m = np.zeros((P, NQ, 4, 7, Z), w1c.dtype)
        w1m[:, :, :, :6, :] = (
            w1c[:, :XMAIN].reshape(NQ, 4, P, 6, Z).transpose(2, 0, 1, 3, 4)
        )
        w1m = np.ascontiguousarray(w1m.reshape(P, NQ * 4 * 7 * Z))
        # block-diagonal tail weights [68, NQ*128]:
        # rows 17j+p: p<16 -> W1T[4q+j, 768+p, :] at cols 32j..; p=16 -> b1
        w1tl = np.zeros((4 * (XT + 1), NQ, P), w1c.dtype)
        tails = w1c[:, XMAIN:].reshape(NQ, 4, XT, Z)
        b1r = b116[s].reshape(NQ, 4, Z)
        for j in range(4):
            w1tl[(XT + 1) * j : (XT + 1) * j + XT, :, 32 * j : 32 * (j + 1)] = (
                tails[:, j].transpose(1, 0, 2)
            )
            w1tl[(XT + 1) * j + XT, :, 32 * j : 32 * (j + 1)] = b1r[:, j]
        w1tl = np.ascontiguousarray(w1tl.reshape(4 * (XT + 1), NQ * P))

        w2qm = np.zeros((P, NQ, 40), np.float16)
        w2c = w2s[s]  # [GPC, C, Z]
        for j in range(4):
            # w2qm[32j+z, q, 10j+c] = W2[4q+j, c, z] * 0.5/1.1
            w2qm[32 * j : 32 * (j + 1), :, 10 * j : 10 * (j + 1)] = (
                w2c[j::4].transpose(2, 0, 1)
            )
        w2qm = np.ascontiguousarray(w2qm.reshape(P, NQ * 40))

        spc = sp[s].reshape(NQ, 4)  # [NQ, 4]
        b1c = b1[s].astype(np.float32).reshape(NQ, 4, Z)
        sphalf = np.empty((P, NQ), np.float32)
        spb1h = np.empty((P, NQ), np.float32)
        for j in range(4):
            sphalf[32 * j : 32 * (j + 1), :] = (spc[:, j] * (0.5 / (SX * SW)))[None, :]
            spb1h[32 * j : 32 * (j + 1), :] = (
                spc[:, j, None] * 0.5 * b1c[:, j]
            ).T
        b2c = b2[s].astype(np.float32).reshape(NQ, 4, C)
        b2qm = np.ascontiguousarray(
            b2c.transpose(1, 2, 0).reshape(40, NQ)
        )

        in_maps.append(
            {
                "xm": xm,
                "xt6": xt6,
                "w1m": w1m,
                "w1tail": w1tl,
                "w2q": w2qm,
                "sphalf": sphalf,
                "spb1h": spb1h,
                "b2q": b2qm,
                "bo1": bo1,
                "bo2": bo2,
            }
        )
    return in_maps


def _run(in_maps, cfg=DEFAULT_CFG, trace=False, tmpdir=None):
    key = str(sorted(cfg.items()))
    if key not in _CACHE:
        _CACHE[key] = _build(cfg)
    return run_bass_kernel_spmd(
        _CACHE[key],
        in_maps,
        core_ids=list(range(NCORE)),
        trace=trace,
        tmpdir=tmpdir,
    )


_LAST = {}


def kernel(x, W1, b1, beta, W2, b2):
    in_maps = _marshal(x, W1, b1, beta, W2, b2)
    trace = bool(os.environ.get("KERNEL_TRACE"))
    r = _run(in_maps, trace=trace, tmpdir=os.environ.get("KERNEL_TRACE_DIR"))
    _LAST["results"] = r
    outs = []
    for c in range(NCORE):
        o = r.results[c]["out"]  # [NQ, 40, L]
        outs.append(
            o.reshape(NQ, 4, C, L).reshape(GPC, C, L).reshape(GPC, C * L)
        )
    return np.concatenate(outs, axis=0).astype(np.float32)
